# revision 41
# baseline (speedup 1.0000x reference)
"""BiMamba2D (VMamba-style 4-direction selective scan) Trainium2 Bass kernel.

Sharding: 8 cores = 4 batches x 2 scan layouts (hw / wh).  The wh layout is
realized by host-transposing the input image (and swapping the conv kernel's
spatial taps), so every core runs the same SPMD program.  Each core computes
both time directions (forward + reversed APs) of its layout and emits a
partial (L, 96) output; the host sums partials (gating and the output
projection are linear across the four direction contributions).

v3 design (all-bf16 matmul/elementwise dataflow, fp32 PSUM):
 - front: PE-transpose x, input projection, 3x3 conv (as 18 accumulating
   matmuls over a zero-padded bf16 image), z-projection; conv/z silu ops are
   adjacent so the ACT Silu table loads once.  B/C projections produce the
   16->128 partition-replicated b_rep/c_rep directly (host-tiled weights).
   delta = softplus(dt-proj + bias) via Exp+Ln (single ACT table, see _Bacc)
   and du = delta*xc are precomputed for all 192 channels over full L.
 - scan phase, j-outer (24 d-blocks of 8 channels x 16 states):
   delta/du rows are partition-replicated 16x by SBUF->SBUF DMA (no PE, no
   PSUM), dA = exp(A*rep_del) is ONE full-L ACT op, dBu = rep_du*b_rep ONE
   full-L DVE op, and each direction's recurrence is ONE full-L
   tensor_tensor_scan (fp32 internal state; the reverse direction just reads
   the same dA/dBu through reversed APs).  y contraction over the 16 states
   runs on PE into 16 PSUM accumulators ([64,512] f/r pairs in 8 banks) that
   live across the whole j loop.  o = h*C sits on GpSimd to keep DVE free
   for the scans (DVE is the critical engine at ~2 cycles/scan-element).
 - tail: y = y_fwd + rev(y_rev) + D*xc, gate with the precomputed silu(z),
   output projection, DMA out in fp32.
"""

import os
import sys
from contextlib import ExitStack

import numpy as np

for _p in ("/opt/trn_rl_repo",):
    if _p not in sys.path and os.path.isdir(_p):
        sys.path.append(_p)

import concourse.bass as bass
import concourse.tile as tile
from concourse import bacc, mybir

F32 = mybir.dt.float32
BF16 = mybir.dt.bfloat16
AL = mybir.AluOpType
AF = mybir.ActivationFunctionType

# Problem constants
B, H, W, CM = 4, 64, 64, 96
L = H * W  # 4096
D = 192  # d_inner
N = 16  # d_state
RK = 6  # dt_rank
TC = 512  # time-chunk (PSUM bank width in fp32)
NCH = L // TC  # 8
NG = 3  # groups of 64 channels
GDB = 8  # d-blocks per group (8 channels each)
HS = [128, 64]  # d_inner row split
HOF = [0, 128]  # absolute channel offset per half
# group -> (half index, row offset within half)
GMAP = [(0, 0), (0, 64), (1, 0)]
WP = W + 2  # padded row stride for conv


def _rev(ap):
    """Reverse an AP along its last (free) dim."""
    return ap[:, ::-1]


def _chain(nc, rep, src_rows):
    """16x partition replication by doubling: 8 -> 16 -> 32 -> 64 -> 128."""
    nc.gpsimd.dma_start(rep[0:8, :], src_rows)
    nc.gpsimd.dma_start(rep[8:16, :], rep[0:8, :])
    nc.gpsimd.dma_start(rep[16:32, :], rep[0:16, :])
    nc.gpsimd.dma_start(rep[32:64, :], rep[0:32, :])
    nc.gpsimd.dma_start(rep[64:128, :], rep[0:64, :])


def build_kernel(ctx: ExitStack, tc: "tile.TileContext", io: dict):
    nc = tc.nc

    # ---------------- weight / constant loads ----------------
    wpool = ctx.enter_context(tc.tile_pool(name="wpool", bufs=1))
    ppool = ctx.enter_context(tc.tile_pool(name="persist", bufs=1))


    # all matmul weights arrive pre-transposed & contiguous from the host
    # (strided DRAM transposes cost 6-13us per DMA on the xbar-less path).
    w_int = wpool.tile([96, 384], BF16, name="w_int")
    nc.sync.dma_start(w_int[:], io["w_inT"])

    xpb_t, xpc_t, dtw_t = [], [], []
    for hh in range(2):
        hsl = slice(HOF[hh], HOF[hh] + HS[hh])
        t = wpool.tile([HS[hh], 128], BF16, name=f"xpb_t{hh}")
        nc.sync.dma_start(t[:], io["xpbT"][hsl, :])
        xpb_t.append(t)
        t = wpool.tile([HS[hh], 128], BF16, name=f"xpc_t{hh}")
        nc.sync.dma_start(t[:], io["xpcT"][hsl, :])
        xpc_t.append(t)
        t = wpool.tile([HS[hh], 192], BF16, name=f"dtw_t{hh}")
        nc.sync.dma_start(t[:], io["dtwT"][hsl, :])
        dtw_t.append(t)

    wout_t = []
    for hh in range(2):
        t = wpool.tile([HS[hh], 96], BF16, name=f"wout_t{hh}")
        nc.sync.dma_start(t[:], io["woutT"][HOF[hh] : HOF[hh] + HS[hh], :])
        wout_t.append(t)

    def vec_col(name):
        tiles = []
        for hh in range(2):
            t = wpool.tile([HS[hh], 1], F32, name=f"{name}{hh}")
            nc.sync.dma_start(
                t[:],
                io[name][HOF[hh] : HOF[hh] + HS[hh]].rearrange("(p one) -> p one", one=1),
            )
            tiles.append(t)
        return tiles

    dtb = vec_col("dt_proj_b")
    convb = vec_col("conv_b")
    d2 = vec_col("d2")

    a_dn = wpool.tile([128, 3 * GDB], F32, name="a_dn")
    nc.sync.dma_start(a_dn[:], io["a_dn"][:])
    rt64 = []  # [j]: [128, 64] n-contraction lhsT into rows j*8..j*8+8
    for j in range(GDB):
        t2 = wpool.tile([128, 64], BF16, name=f"rt64_{j}")
        nc.sync.dma_start(t2[:], io["rt64"][j])
        rt64.append(t2)
    # ---------------- persistent big buffers (bf16) ----------------
    xc = [ppool.tile([HS[hh], L], BF16, name=f"xc{hh}") for hh in range(2)]
    y_sb = [ppool.tile([HS[hh], L], BF16, name=f"y{hh}") for hh in range(2)]
    zs = [ppool.tile([HS[hh], L], BF16, name=f"zs{hh}") for hh in range(2)]
    b_rep = ppool.tile([128, L], BF16, name="b_rep")
    c_rep = ppool.tile([128, L], BF16, name="c_rep")
    dlf = [ppool.tile([HS[hh], L], BF16, name=f"dlf{hh}") for hh in range(2)]
    duf = [ppool.tile([HS[hh], L], BF16, name=f"duf{hh}") for hh in range(2)]

    xT = ppool.tile([96, L], BF16, name="xT")  # x transposed (ch, t)

    # ============ phase 1+2: transpose x, in-proj, conv, silus ============
    with (
        tc.tile_pool(name="padpool", bufs=1) as padpool,
        tc.tile_pool(name="cwpool", bufs=1) as cwpool,
    ):
        # fused conv weights: conv(W_in_x @ x) == conv_eff(x) with
        # W_eff[o,i,tap] = sum_c conv_w[o,c,tap] * W_in[c,i]; contraction is
        # over the 96 raw input channels, halving the conv matmul count and
        # eliminating the separate input-projection pass.
        cw = {}
        for oh in range(2):
            for kh in range(3):
                for kw in range(3):
                    t = cwpool.tile([96, HS[oh]], BF16, name=f"cw{oh}{kh}{kw}")
                    k = oh * 9 + kh * 3 + kw
                    nc.sync.dma_start(t[:], io["cw_pack"][k, 0:96, 0 : HS[oh]])
                    cw[(oh, kh, kw)] = t

        xpad = padpool.tile([96, (H + 2) * WP], BF16, name="xpad")
        nc.gpsimd.memset(xpad[:], 0.0)

        # x arrives host-transposed [96, L]; contiguous chunk loads, then the
        # padded conv image is filled by ACT copies (cheap, off the PE).
        for ch in range(NCH):
            tsl = slice(ch * TC, (ch + 1) * TC)
            nc.sync.dma_start(xT[:, tsl], io["x"][:, tsl])
            dst = (
                xpad[:]
                .rearrange("p (h w) -> p h w", w=WP)[
                    :, ch * 8 + 1 : ch * 8 + 9, 1 : W + 1
                ]
            )
            nc.scalar.copy(dst, xT[:, tsl])

        # 3x3 conv + bias + silu
        with tc.tile_pool(name="p2ps", bufs=1, space="PSUM") as p2ps:
            for ch in range(NCH):
                tsl = slice(ch * TC, (ch + 1) * TC)
                for oh in range(2):
                    ps = p2ps.tile([HS[oh], TC], F32, tag=f"ps_cv{oh}", bufs=2)
                    for kh in range(3):
                        for kw in range(3):
                            rhs = (
                                xpad[:]
                                .rearrange("p (h w) -> p h w", w=WP)[
                                    :, ch * 8 + kh : ch * 8 + kh + 8, kw : kw + W
                                ]
                            )
                            nc.tensor.matmul(
                                ps[:],
                                cw[(oh, kh, kw)][:],
                                rhs,
                                start=(kh == 0 and kw == 0),
                                stop=(kh == 2 and kw == 2),
                            )
                    nc.scalar.activation(
                        xc[oh][:, tsl], ps[:], AF.Silu, bias=convb[oh][:, 0:1]
                    )

    # ============ phase 3: B/C projection + delta/du precompute ============
    with (
        tc.tile_pool(name="p3ps", bufs=2, space="PSUM") as p3ps,
        tc.tile_pool(name="e1pool", bufs=3) as e1pool,
    ):
        def delta_chunk(ch, oh):
            tsl = slice(ch * TC, (ch + 1) * TC)
            ps_d = p3ps.tile([HS[oh], TC], F32, tag=f"ps_dt{oh}", name="ps_d")
            nc.tensor.matmul(
                ps_d[:], dtw_t[0][:, HOF[oh] : HOF[oh] + HS[oh]],
                xc[0][:, tsl], start=True, stop=False,
            )
            nc.tensor.matmul(
                ps_d[:], dtw_t[1][:, HOF[oh] : HOF[oh] + HS[oh]],
                xc[1][:, tsl], start=False, stop=True,
            )
            e1 = e1pool.tile([HS[oh], TC], BF16, tag=f"e1{oh}", name="e1")
            nc.scalar.activation(e1[:], ps_d[:], AF.Exp, bias=dtb[oh][:, 0:1])
            # delta = ln(1 + e1)  (softplus)
            nc.scalar.activation(dlf[oh][:, tsl], e1[:], AF.Ln, bias=1.0)
            nc.vector.tensor_tensor(
                duf[oh][:, tsl], dlf[oh][:, tsl], xc[oh][:, tsl], AL.mult
            )

        for ch in range(NCH):
            tsl = slice(ch * TC, (ch + 1) * TC)
            ps_b = p3ps.tile([128, TC], F32, tag="ps_bc")
            nc.tensor.matmul(ps_b[:], xpb_t[0][:], xc[0][:, tsl], start=True, stop=False)
            nc.tensor.matmul(ps_b[:], xpb_t[1][:], xc[1][:, tsl], start=False, stop=True)
            nc.scalar.copy(b_rep[:, tsl], ps_b[:])
            ps_c = p3ps.tile([128, TC], F32, tag="ps_bc")
            nc.tensor.matmul(ps_c[:], xpc_t[0][:], xc[0][:, tsl], start=True, stop=False)
            nc.tensor.matmul(ps_c[:], xpc_t[1][:], xc[1][:, tsl], start=False, stop=True)
            nc.scalar.copy(c_rep[:, tsl], ps_c[:])
            delta_chunk(ch, 0)
            delta_chunk(ch, 1)

    # ---- z-projection + silu (needed only by phase 5; overlaps the scan) ----
    with tc.tile_pool(name="zps", bufs=2, space="PSUM") as zps:
        for ch in range(NCH):
            tsl = slice(ch * TC, (ch + 1) * TC)
            for oh in range(2):
                ps_z = zps.tile([HS[oh], TC], F32, tag=f"ps_z{oh}")
                nc.tensor.matmul(
                    ps_z[:],
                    w_int[:, 192 + HOF[oh] : 192 + HOF[oh] + HS[oh]],
                    xT[:, tsl],
                    start=True,
                    stop=True,
                )
                nc.scalar.activation(zs[oh][:, tsl], ps_z[:], AF.Silu)

    # ================= phase 4: selective scan (fwd + rev) =================
    with (
        tc.tile_pool(name="reppool", bufs=2) as reppool,
        tc.tile_pool(name="jpool", bufs=2) as jpool,
        tc.tile_pool(name="hpool", bufs=1) as hpool,
        tc.tile_pool(name="opool", bufs=1) as opool,
    ):
        for g in range(NG):
            hh, gr0 = GMAP[g]
            gp = slice(gr0, gr0 + 64)
            with tc.tile_pool(name=f"scps{g}", bufs=1, space="PSUM") as scps:
                # ch 0-3 in rows 0:64 / ch 4-7 in rows 64:128 of 4 banks;
                # fwd and rev(read back in fwd coords) share one accumulator.
                accb = [
                    scps.tile([128, TC], F32, tag=f"acc{b}", name=f"acc{g}_{b}")
                    for b in range(4)
                ]
                acc = [
                    accb[ch % 4][(ch // 4) * 64 : (ch // 4) * 64 + 64, :]
                    for ch in range(NCH)
                ]
                for j in range(GDB):
                    db = g * GDB + j
                    rows = slice(gr0 + j * 8, gr0 + j * 8 + 8)

                    # n-major replication (partition p holds ch j*8 + p%8,
                    # state p//8) by partition-doubling SBUF->SBUF DMAs.
                    rep_del = reppool.tile([128, L], BF16, tag="rep_del")
                    rep_du = reppool.tile([128, L], BF16, tag="rep_du")
                    _chain(nc, rep_del, dlf[hh][rows, :])
                    _chain(nc, rep_du, duf[hh][rows, :])

                    dA = jpool.tile([128, L], BF16, tag="dA")
                    nc.scalar.activation(
                        dA[:], rep_del[:], AF.Exp, scale=a_dn[:, db : db + 1]
                    )
                    dBu = jpool.tile([128, L], BF16, tag="dBu")
                    nc.vector.tensor_tensor(dBu[:], rep_du[:], b_rep[:], AL.mult)

                    h_f = hpool.tile([128, L], BF16, tag="h_f")
                    nc.vector.tensor_tensor_scan(
                        h_f[:], dA[:], dBu[:], 0.0, AL.mult, AL.add
                    )
                    h_r = hpool.tile([128, L], BF16, tag="h_r")
                    nc.vector.tensor_tensor_scan(
                        h_r[:], _rev(dA[:]), _rev(dBu[:]), 0.0, AL.mult, AL.add
                    )

                    # o = h * C on DVE (bf16 2x mode; GpSimd would stall on
                    # the SBUF port it shares with the always-busy DVE).  The
                    # rev h is read back through a reversed AP so o_r lands in
                    # forward time coordinates and shares the fwd accumulator.
                    o_f = opool.tile([128, L], BF16, tag="o_f")
                    nc.vector.tensor_tensor(o_f[:], h_f[:], c_rep[:], AL.mult)
                    o_r = opool.tile([128, L], BF16, tag="o_r")
                    nc.vector.tensor_tensor(
                        o_r[:], _rev(h_r[:]), c_rep[:], AL.mult
                    )
                    for ch in range(NCH):
                        csl = slice(ch * TC, (ch + 1) * TC)
                        nc.tensor.matmul(
                            acc[ch], rt64[j][:], o_f[:, csl],
                            start=(j == 0), stop=False,
                        )
                        nc.tensor.matmul(
                            acc[ch], rt64[j][:], o_r[:, csl],
                            start=False, stop=(j == GDB - 1),
                        )

                # drain: y = y_fwd + y_rev (already summed in PSUM).  For
                # g0/g1 the copies run on DVE, which is stalled at the group
                # boundary anyway, so ACT can start the next group's dA exp
                # immediately; g2's run on ACT to keep DVE free for phase 5.
                for ch in range(NCH):
                    tsl = slice(ch * TC, (ch + 1) * TC)
                    if g < 2 and ch < 5:
                        # fill the DVE boundary stall while ACT runs the next
                        # group's dA exp; the rest go back to ACT once it is
                        # free again.
                        nc.vector.tensor_copy(y_sb[hh][gp, tsl], acc[ch])
                    else:
                        nc.scalar.copy(y_sb[hh][gp, tsl], acc[ch])

    # ================= phase 5: D*u, gate with silu(z), out-proj =================
    with (
        tc.tile_pool(name="p6ps", bufs=2, space="PSUM") as p6ps,
        tc.tile_pool(name="p6sb", bufs=3) as p6sb,
    ):
        for ch in range(NCH):
            tsl = slice(ch * TC, (ch + 1) * TC)
            yg = []
            for hh in range(2):
                yf = p6sb.tile([HS[hh], TC], BF16, tag=f"yf{hh}")
                nc.vector.scalar_tensor_tensor(
                    yf[:], xc[hh][:, tsl], d2[hh][:, 0:1], y_sb[hh][:, tsl],
                    AL.mult, AL.add,
                )
                g = p6sb.tile([HS[hh], TC], BF16, tag=f"yg{hh}")
                nc.gpsimd.tensor_tensor(g[:], yf[:], zs[hh][:, tsl], AL.mult)
                yg.append(g)

            # transposed orientation: out.T[96, t] = woutT.T @ yg -- tokens on
            # the moving dim, so one N=512 matmul pair per chunk instead of
            # four N=96 pairs, and a single output DMA; host un-transposes.
            ps_o = p6ps.tile([96, TC], F32, tag="ps_o")
            nc.tensor.matmul(ps_o[:], wout_t[0][:], yg[0][:], start=True, stop=False)
            nc.tensor.matmul(ps_o[:], wout_t[1][:], yg[1][:], start=False, stop=True)
            stg = p6sb.tile([96, TC], F32, tag="stg")
            nc.scalar.copy(stg[:], ps_o[:])
            nc.sync.dma_start(io["out"][:, tsl], stg[:])


# ---------------------------------------------------------------------------
# host-side wrapper
# ---------------------------------------------------------------------------

def _bf16(a):
    import ml_dtypes

    return np.ascontiguousarray(np.asarray(a, np.float32).astype(ml_dtypes.bfloat16))


def _host_constants(A_logs):
    # n-major scan layout: partition p of a d-block tile holds
    # channel db*8 + p%8, state p//8.
    A = -np.exp(np.asarray(A_logs, np.float32))  # (192, 16)
    p = np.arange(128)
    a_dn = np.zeros((128, 3 * GDB), np.float32)
    for db in range(3 * GDB):
        a_dn[:, db] = A[db * 8 + p % 8, p // 8]
    rt64 = np.zeros((GDB, 128, 64), np.float32)
    for j in range(GDB):
        rt64[j] = (j * 8 + p % 8)[:, None] == np.arange(64)[None, :]
    ident = np.eye(128, dtype=np.float32)
    return a_dn, rt64, ident


class _Bacc(bacc.Bacc):
    """Bacc with activation-table preference adjusted.

    The stock first-fit table assignment maps Exp -> exp_and_others and
    Ln -> natural_log, so every Exp/Ln pair swaps the ACT table (1.28 us
    per swap).  Blanking those two tables makes first-fit pick
    natural_log_exp_and_others (which holds both) at its canonical index."""

    def insert_act_table_loads(self):
        import bass_rust as _bass_rust
        from concourse.hw_specs import get_activation_tables

        has_activation = any(
            isinstance(i, mybir.InstActivation)
            for b in self.main_func.blocks
            for i in b.instructions
        )
        if not has_activation:
            return
        tables = list(get_activation_tables(self.m.arch).items())
        tables = [
            (name, set() if name in ("exp_and_others", "natural_log") else fns)
            for name, fns in tables
        ]
        _bass_rust.insert_act_table_loads(self, tables)


_NC_CACHE = {}


def _get_nc():
    if "nc" in _NC_CACHE:
        return _NC_CACHE["nc"]
    nc = _Bacc(
        "TRN2", target_bir_lowering=False, debug=False, enable_asserts=False,
        num_devices=8,
    )
    io = {
        "x": nc.dram_tensor("x", [CM, L], BF16, kind="ExternalInput").ap(),
        "w_inT": nc.dram_tensor("w_inT", [CM, 2 * D], BF16, kind="ExternalInput").ap(),
        "cw_pack": nc.dram_tensor("cw_pack", [18, 96, 128], BF16, kind="ExternalInput").ap(),
        "conv_b": nc.dram_tensor("conv_b", [D], F32, kind="ExternalInput").ap(),
        "xpbT": nc.dram_tensor("xpbT", [D, 128], BF16, kind="ExternalInput").ap(),
        "xpcT": nc.dram_tensor("xpcT", [D, 128], BF16, kind="ExternalInput").ap(),
        "dtwT": nc.dram_tensor("dtwT", [D, D], BF16, kind="ExternalInput").ap(),
        "dt_proj_b": nc.dram_tensor("dt_proj_b", [D], F32, kind="ExternalInput").ap(),
        "d2": nc.dram_tensor("d2", [D], F32, kind="ExternalInput").ap(),
        "woutT": nc.dram_tensor("woutT", [D, CM], BF16, kind="ExternalInput").ap(),
        "a_dn": nc.dram_tensor("a_dn", [128, 3 * GDB], F32, kind="ExternalInput").ap(),
        "rt64": nc.dram_tensor("rt64", [GDB, 128, 64], BF16, kind="ExternalInput").ap(),
        "ident": nc.dram_tensor("ident", [128, 128], BF16, kind="ExternalInput").ap(),
        "out": nc.dram_tensor("out", [CM, L], F32, kind="ExternalOutput").ap(),
    }
    with tile.TileContext(nc) as tc:
        with ExitStack() as ctx:
            build_kernel(ctx, tc, io)
    nc.compile()
    _NC_CACHE["nc"] = nc
    _NC_CACHE["io_names"] = list(io.keys())
    return nc


def make_in_maps(x, W_in, conv_w, conv_b, x_proj_w, dt_proj_w, dt_proj_b, A_logs,
                 Ds, W_out):
    f = lambda a: np.ascontiguousarray(np.asarray(a, dtype=np.float32))
    a_dn, rt64, ident = _host_constants(A_logs)
    xpw = f(x_proj_w)
    common = {
        "w_inT": _bf16(f(W_in).T), "conv_b": f(conv_b),
        "xpbT": _bf16(np.repeat(xpw[RK : RK + N], 8, axis=0).T),
        "xpcT": _bf16(np.repeat(xpw[RK + N : RK + 2 * N], 8, axis=0).T),
        "dtwT": _bf16((f(dt_proj_w) @ xpw[:RK]).T),
        "dt_proj_b": f(dt_proj_b),
        "d2": f(Ds) * 2.0, "woutT": _bf16(f(W_out).T), "a_dn": a_dn,
        "rt64": _bf16(rt64), "ident": _bf16(ident),
    }

    w_in_x = f(W_in)[:D, :]  # (192, 96): the xp half of the input projection

    def pack_conv(cwa):
        # fuse the input projection into the conv: W_eff[o,i,kh,kw]
        w_eff = np.einsum("ockl,ci->oikl", cwa, w_in_x, optimize=True)
        pack = np.zeros((18, 96, 128), np.float32)
        for oh in range(2):
            for kh in range(3):
                for kw in range(3):
                    k = oh * 9 + kh * 3 + kw
                    blk = w_eff[HOF[oh] : HOF[oh] + HS[oh], :, kh, kw].T
                    pack[k, :, 0 : HS[oh]] = blk
        return _bf16(pack)

    x = f(x)
    cw = f(conv_w)
    cw_t = np.ascontiguousarray(cw.transpose(0, 1, 3, 2))
    pk, pk_t = pack_conv(cw), pack_conv(cw_t)
    in_maps = []
    for c in range(8):
        b, lay = c // 2, c % 2
        xv = x[b] if lay == 0 else np.ascontiguousarray(x[b].transpose(1, 0, 2))
        in_maps.append(
            {**common, "x": _bf16(xv.reshape(L, CM).T),
             "cw_pack": pk if lay == 0 else pk_t}
        )
    return in_maps


def assemble(parts):
    out = np.zeros((B, L, CM), np.float32)
    for c in range(8):
        b, lay = c // 2, c % 2
        p = np.asarray(parts[c], np.float32).T
        if lay:
            p = p.reshape(W, H, CM).transpose(1, 0, 2).reshape(L, CM)
        out[b] += p
    return out.reshape(B, H, W, CM)


def kernel(**inputs):
    from concourse.bass_utils import run_bass_kernel_spmd

    nc = _get_nc()
    in_maps = make_in_maps(**inputs)
    res = run_bass_kernel_spmd(nc, in_maps, list(range(8)))
    return assemble([res.results[c]["out"] for c in range(8)])
